# revision 5
# baseline (speedup 1.0000x reference)
"""DeepseekV3 MoE experts kernel for 8 Trainium2 NeuronCores.

Problem: every expert processes the FULL token set.
  g = x @ w_gate[e].T ; u = x @ w_up[e].T ; h = silu(g)*u
  out[e] = h @ w_down[e].T ;  concat over e -> [E*T, H]

Sharding: expert-parallel. Core c owns experts {2c, 2c+1}; hidden_states is
replicated; outputs are concatenated host-side (no on-device collectives).

Per-core compute (per expert e, with xT = x.T resident in SBUF):
  phase 1: gT[m*128:(m+1)*128, :] = wg_slab[m][:,k,:].T @ xT[:,k,:] (acc over k)
  phase 2: same for uT; hT = silu(gT) * uT  (in [I, T] layout, no transposes)
  phase 3: out[mt*128.., nslice] = hT[:,k,mtslice].T @ wd_slab[n][:,k,:]

All matmul operands are bf16 (host-cast); PSUM accumulation is fp32 and the
output is written back in fp32.

DMA plan (the mid-kernel PE stream is already at roofline; the wins are at
the edges):
  - Weights are laid out host-side in consumption-order slabs: wg/wu as
    [E, MO, 128, KO, 128] m-major slabs (0.5 MiB each; phase-1 m-chunk m
    needs only slab m), wd as [E, NH, 128, MO, 512] n-major slabs (1 MiB).
    The very first slab is split so the first matmul is gated on ~160 KiB.
  - All weight DMAs go on the Sync HWDGE ring in exact consumption order.
  - xT and all output stores go on the Scalar HWDGE ring (the second
    physical HWDGE ring), so the x ramp runs in parallel with the weight
    ramp and output stores never head-of-line block weight prefetches.
  - PSUM->SBUF output copies run on the (otherwise idle) Vector engine.
  - The last output group is split in two so the final store's HBM write
    receipt (which gates the kernel drain) covers half the bytes.

Self-contained: shapes hardcoded; inputs are the full arrays from
setup_inputs(); returns the full [4096, 2048] fp32 output.
"""

import numpy as np
import ml_dtypes

E, T, H, I = 16, 256, 2048, 1024
N_CORES = 8
E_PER = E // N_CORES  # 2
P = 128
KO = H // P  # 16 k-chunks for phases 1/2
MO = I // P  # 8 m-chunks for phases 1/2 (= k-chunks for phase 3)
TO = T // P  # 2 m-chunks for phase 3
NS = 512  # n-slice width for phase 3
NH = H // NS  # 4

WARMUP_MM = 16  # ~3.4us of cold matmuls bridging the preamble to first data;
# sized so the HAM clock gate flips to 8/8 right as the first weights land

_CACHE: dict = {}


def _build_program(sim_compat=False):
    # sim_compat: CoreSim lacks the Silu LUT — express silu as
    # sigmoid(g)*g with an extra DVE multiply. HW uses the fused Silu op.
    import concourse.mybir as mybir
    import concourse.tile as tile
    from concourse import bacc

    dt = mybir.dt.bfloat16
    f32 = mybir.dt.float32
    AF = mybir.ActivationFunctionType

    nc = bacc.Bacc(None, target_bir_lowering=False, debug=False)

    xT = nc.dram_tensor("xT", [P, KO, T], dt, kind="ExternalInput")[:]
    wg = nc.dram_tensor("wg", [E_PER, MO, P, KO, P], dt, kind="ExternalInput")[:]
    wu = nc.dram_tensor("wu", [E_PER, MO, P, KO, P], dt, kind="ExternalInput")[:]
    wd = nc.dram_tensor("wd", [E_PER, NH, P, MO, NS], dt, kind="ExternalInput")[:]
    out = nc.dram_tensor("out", [E_PER, TO, P, H], f32, kind="ExternalOutput")[:]

    with tile.TileContext(nc) as tc:
        with (
            tc.tile_pool(name="xp", bufs=1) as xp,
            tc.tile_pool(name="wgp", bufs=10) as wgp,
            tc.tile_pool(name="wup", bufs=9) as wup,
            tc.tile_pool(name="wdp", bufs=6) as wdp,
            tc.tile_pool(name="hp", bufs=2) as hp,
            tc.tile_pool(name="gp", bufs=8) as gp,
            tc.tile_pool(name="op", bufs=8) as outp,
            tc.tile_pool(name="ps", bufs=8, space="PSUM") as ps,
        ):
            # PE warm-up on a zeroed scratch tile: flips the HAM clock gate
            # toward 8/8 while the first input DMAs are still in flight.
            warm = xp.tile([P, T], dt, tag="warm")
            nc.vector.memset(warm[:], 0.0)
            wps = ps.tile([P, T], f32, tag="ps")
            for _ in range(WARMUP_MM):
                nc.tensor.matmul(wps[:], warm[:, :P], warm[:], start=True, stop=True)

            xtiles = []  # (k0, kq, tile)
            wslabs: dict = {}  # (which, e, m) -> list of (k0, kq, tile)
            wdslabs: dict = {}  # (e, n) -> tile

            def issue_x(k0, kq, tag):
                t_ = xp.tile([P, kq, T], dt, tag=tag)
                nc.scalar.dma_start(t_[:], xT[:, k0 : k0 + kq, :])
                xtiles.append((k0, kq, t_))

            def xslice(k):
                for k0, kq, t_ in xtiles:
                    if k0 <= k < k0 + kq:
                        return t_[:, k - k0, :]
                raise KeyError(k)

            def issue_w(pool, src, which, e, m, k0, kq):
                t_ = pool.tile([P, kq, P], dt, tag=pool.name)
                nc.sync.dma_start(t_[:], src[e, m, :, k0 : k0 + kq, :])
                wslabs.setdefault((which, e, m), []).append((k0, kq, t_))

            def wslice(which, e, m, k):
                for k0, kq, t_ in wslabs[(which, e, m)]:
                    if k0 <= k < k0 + kq:
                        return t_[:, k - k0, :]
                raise KeyError((which, e, m, k))

            def issue_wd(e, n):
                t_ = wdp.tile([P, MO, NS], dt, tag="wd")
                nc.sync.dma_start(t_[:], wd[e, n, :, :, :])
                wdslabs[(e, n)] = t_

            # DMA issue order == consumption order. Weights on the Sync
            # ring; x on the Scalar ring so both ramps run in parallel.
            # The leading pieces are small so the first matmul starts as
            # soon as ~160 KiB has landed.
            issue_x(0, 8, "xa")
            issue_w(wgp, wg, "wg", 0, 0, 0, KO)
            issue_x(8, 8, "xb")
            for m in range(1, MO):
                issue_w(wgp, wg, "wg", 0, m, 0, KO)
            for m in range(MO):
                issue_w(wup, wu, "wu", 0, m, 0, KO)
            for n in range(NH):
                issue_wd(0, n)
            for e in range(1, E_PER):
                for m in range(MO):
                    issue_w(wgp, wg, "wg", e, m, 0, KO)
                for m in range(MO):
                    issue_w(wup, wu, "wu", e, m, 0, KO)
                for n in range(NH):
                    issue_wd(e, n)

            for e in range(E_PER):
                hT = hp.tile([P, MO, T], dt, tag="h")
                # all gate m-chunks first: the PE is gated only by the wg
                # stream, never by wu (which lands while these run)
                gss = []
                for m in range(MO):
                    pg = ps.tile([P, T], f32, tag="ps")
                    for k in range(KO):
                        nc.tensor.matmul(
                            pg[:],
                            wslice("wg", e, m, k),
                            xslice(k),
                            start=(k == 0),
                            stop=(k == KO - 1),
                        )
                    gs = gp.tile([P, T], f32, tag="g")
                    if sim_compat:
                        nc.scalar.activation(gs[:], pg[:], AF.Sigmoid)
                        nc.vector.tensor_tensor(
                            gs[:], gs[:], pg[:], mybir.AluOpType.mult
                        )
                    else:
                        nc.scalar.activation(gs[:], pg[:], AF.Silu)
                    gss.append(gs)
                for m in range(MO):
                    pu = ps.tile([P, T], f32, tag="ps")
                    for k in range(KO):
                        nc.tensor.matmul(
                            pu[:],
                            wslice("wu", e, m, k),
                            xslice(k),
                            start=(k == 0),
                            stop=(k == KO - 1),
                        )
                    nc.vector.tensor_tensor(
                        hT[:, m, :], gss[m][:], pu[:], mybir.AluOpType.mult
                    )
                for n in range(NH):
                    wdt = wdslabs[(e, n)]
                    for mt in range(TO):
                        last = e == E_PER - 1 and n == NH - 1 and mt == TO - 1
                        # the final group is stored as two half-width
                        # slices so the last HBM write receipt is smaller
                        nsub = 2 if last else 1
                        sub = NS // nsub
                        for s in range(nsub):
                            po = ps.tile([P, sub], f32, tag="ps")
                            for k in range(MO):
                                nc.tensor.matmul(
                                    po[:],
                                    hT[:, k, mt * P : (mt + 1) * P],
                                    wdt[:, k, s * sub : (s + 1) * sub],
                                    start=(k == 0),
                                    stop=(k == MO - 1),
                                )
                            ot = outp.tile([P, sub], f32, tag="o")
                            nc.vector.tensor_copy(ot[:], po[:])
                            nc.scalar.dma_start(
                                out[e, mt, :, n * NS + s * sub : n * NS + (s + 1) * sub],
                                ot[:],
                            )

    nc.compile()
    return nc


def get_program(sim_compat=False):
    key = ("nc", sim_compat)
    if key not in _CACHE:
        _CACHE[key] = _build_program(sim_compat=sim_compat)
    return _CACHE[key]


def _prep_in_maps(hidden_states, w_gate, w_up, w_down):
    bf16 = ml_dtypes.bfloat16
    x = np.asarray(hidden_states, dtype=np.float32)
    wg = np.asarray(w_gate, dtype=np.float32)
    wu = np.asarray(w_up, dtype=np.float32)
    wd = np.asarray(w_down, dtype=np.float32)

    # xT: [H, T] -> [128, KO, T], partition p + chunk k <-> H index k*128+p
    xt = np.ascontiguousarray(
        x.T.reshape(KO, P, T).transpose(1, 0, 2).astype(bf16)
    )
    # w_gate/w_up: [E, I, H] -> m-major slabs [E, MO, P, KO, 128]:
    #   slab[e, m, p, k, c] = W.T[k*128+p, m*128+c] = w[e, m*128+c, k*128+p]
    wgt = np.ascontiguousarray(
        wg.reshape(E, MO, P, KO, P).transpose(0, 1, 4, 3, 2).astype(bf16)
    )
    wut = np.ascontiguousarray(
        wu.reshape(E, MO, P, KO, P).transpose(0, 1, 4, 3, 2).astype(bf16)
    )
    # w_down: [E, H, I] -> n-major slabs [E, NH, P, MO, NS]:
    #   slab[e, n, p, k, c] = W.T[k*128+p, n*512+c] = w[e, n*512+c, k*128+p]
    wdt = np.ascontiguousarray(
        wd.reshape(E, NH, NS, MO, P).transpose(0, 1, 4, 3, 2).astype(bf16)
    )

    in_maps = []
    for c in range(N_CORES):
        sl = slice(c * E_PER, (c + 1) * E_PER)
        in_maps.append(
            {
                "xT": xt,
                "wg": np.ascontiguousarray(wgt[sl]),
                "wu": np.ascontiguousarray(wut[sl]),
                "wd": np.ascontiguousarray(wdt[sl]),
            }
        )
    return in_maps


def kernel(hidden_states, w_gate, w_up, w_down, _trace=False, _trace_kwargs=None):
    from concourse.bass_utils import run_bass_kernel_spmd

    nc = get_program()
    in_maps = _prep_in_maps(hidden_states, w_gate, w_up, w_down)
    kwargs = {}
    if _trace:
        kwargs = dict(trace=True, **(_trace_kwargs or {}))
    res = run_bass_kernel_spmd(nc, in_maps, core_ids=list(range(N_CORES)), **kwargs)
    out = np.concatenate(
        [res.results[c]["out"].reshape(E_PER * T, H) for c in range(N_CORES)], axis=0
    )
    if _trace:
        _CACHE["last_results"] = res
    return out


# revision 6
# speedup vs baseline: 1.0099x; 1.0099x over previous
"""DeepseekV3 MoE experts kernel for 8 Trainium2 NeuronCores.

Problem: every expert processes the FULL token set.
  g = x @ w_gate[e].T ; u = x @ w_up[e].T ; h = silu(g)*u
  out[e] = h @ w_down[e].T ;  concat over e -> [E*T, H]

Sharding: expert-parallel. Core c owns experts {2c, 2c+1}; hidden_states is
replicated; outputs are concatenated host-side (no on-device collectives).

Per-core compute (per expert e, with xT = x.T resident in SBUF):
  phase 1: gT[m*128:(m+1)*128, :] = wg_slab[m][:,k,:].T @ xT[:,k,:] (acc over k)
  phase 2: same for uT; hT = silu(gT) * uT  (in [I, T] layout, no transposes)
  phase 3: out[mt*128.., nslice] = hT[:,k,mtslice].T @ wd_slab[n][:,k,:]

All matmul operands are bf16 (host-cast); PSUM accumulation is fp32 and the
output is written back in fp32.

DMA plan (the mid-kernel PE stream is already at roofline; the wins are at
the edges):
  - Weights are laid out host-side in consumption-order slabs: wg/wu as
    [E, MO, 128, KO, 128] m-major slabs (0.5 MiB each; phase-1 m-chunk m
    needs only slab m), wd as [E, NH, 128, MO, 512] n-major slabs (1 MiB).
    The very first slab is split so the first matmul is gated on ~160 KiB.
  - All weight DMAs go on the Sync HWDGE ring in exact consumption order.
  - xT and all output stores go on the Scalar HWDGE ring (the second
    physical HWDGE ring), so the x ramp runs in parallel with the weight
    ramp and output stores never head-of-line block weight prefetches.
  - PSUM->SBUF output copies run on the (otherwise idle) Vector engine.
  - The last output group is split in two so the final store's HBM write
    receipt (which gates the kernel drain) covers half the bytes.

Self-contained: shapes hardcoded; inputs are the full arrays from
setup_inputs(); returns the full [4096, 2048] fp32 output.
"""

import numpy as np
import ml_dtypes

E, T, H, I = 16, 256, 2048, 1024
N_CORES = 8
E_PER = E // N_CORES  # 2
P = 128
KO = H // P  # 16 k-chunks for phases 1/2
MO = I // P  # 8 m-chunks for phases 1/2 (= k-chunks for phase 3)
TO = T // P  # 2 m-chunks for phase 3
NS = 512  # n-slice width for phase 3
NH = H // NS  # 4

WARMUP_MM = 28  # matmuls on a scratch tile bridging the preamble (~7us) to
# the point where the input stream is deep enough for a gapless phase 1
# (~11.7us). The HAM clock gate flips to 8/8 ~3.4us in, so the real matmul
# stream starts warm and never idles long enough to re-throttle.

_CACHE: dict = {}


def _build_program(sim_compat=False):
    # sim_compat: CoreSim lacks the Silu LUT — express silu as
    # sigmoid(g)*g with an extra DVE multiply. HW uses the fused Silu op.
    import concourse.mybir as mybir
    import concourse.tile as tile
    from concourse import bacc

    dt = mybir.dt.bfloat16
    f32 = mybir.dt.float32
    AF = mybir.ActivationFunctionType

    nc = bacc.Bacc(None, target_bir_lowering=False, debug=False)

    xT = nc.dram_tensor("xT", [P, KO, T], dt, kind="ExternalInput")[:]
    wg = nc.dram_tensor("wg", [E_PER, MO, P, KO, P], dt, kind="ExternalInput")[:]
    wu = nc.dram_tensor("wu", [E_PER, MO, P, KO, P], dt, kind="ExternalInput")[:]
    wd = nc.dram_tensor("wd", [E_PER, NH, P, MO, NS], dt, kind="ExternalInput")[:]
    out = nc.dram_tensor("out", [E_PER, TO, P, H], f32, kind="ExternalOutput")[:]

    with tile.TileContext(nc) as tc:
        with (
            tc.tile_pool(name="xp", bufs=1) as xp,
            tc.tile_pool(name="wgp", bufs=10) as wgp,
            tc.tile_pool(name="wup", bufs=9) as wup,
            tc.tile_pool(name="wdp", bufs=6) as wdp,
            tc.tile_pool(name="hp", bufs=2) as hp,
            tc.tile_pool(name="gp", bufs=8) as gp,
            tc.tile_pool(name="op", bufs=8) as outp,
            tc.tile_pool(name="ps", bufs=8, space="PSUM") as ps,
        ):
            # PE warm-up on a zeroed scratch tile: flips the HAM clock gate
            # toward 8/8 while the first input DMAs are still in flight.
            warm = xp.tile([P, T], dt, tag="warm")
            nc.vector.memset(warm[:], 0.0)
            wps = ps.tile([P, T], f32, tag="ps")
            for _ in range(WARMUP_MM):
                nc.tensor.matmul(wps[:], warm[:, :P], warm[:], start=True, stop=True)

            xtiles = []  # (k0, kq, tile)
            wslabs: dict = {}  # (which, e, m) -> list of (k0, kq, tile)
            wdslabs: dict = {}  # (e, n) -> tile

            def issue_x(k0, kq, tag):
                t_ = xp.tile([P, kq, T], dt, tag=tag)
                nc.scalar.dma_start(t_[:], xT[:, k0 : k0 + kq, :])
                xtiles.append((k0, kq, t_))

            def xslice(k):
                for k0, kq, t_ in xtiles:
                    if k0 <= k < k0 + kq:
                        return t_[:, k - k0, :]
                raise KeyError(k)

            def issue_w(pool, src, which, e, m, k0, kq):
                t_ = pool.tile([P, kq, P], dt, tag=pool.name)
                nc.sync.dma_start(t_[:], src[e, m, :, k0 : k0 + kq, :])
                wslabs.setdefault((which, e, m), []).append((k0, kq, t_))

            def wslice(which, e, m, k):
                for k0, kq, t_ in wslabs[(which, e, m)]:
                    if k0 <= k < k0 + kq:
                        return t_[:, k - k0, :]
                raise KeyError((which, e, m, k))

            def issue_wd(e, n):
                t_ = wdp.tile([P, MO, NS], dt, tag="wd")
                nc.sync.dma_start(t_[:], wd[e, n, :, :, :])
                wdslabs[(e, n)] = t_

            # DMA issue order == consumption order. Weights on the Sync
            # ring; x on the Scalar ring so both ramps run in parallel.
            # The leading pieces are small so the first matmul starts as
            # soon as ~160 KiB has landed.
            issue_x(0, 8, "xa")
            issue_w(wgp, wg, "wg", 0, 0, 0, KO)
            issue_x(8, 8, "xb")
            for m in range(1, MO):
                issue_w(wgp, wg, "wg", 0, m, 0, KO)
            for m in range(MO):
                issue_w(wup, wu, "wu", 0, m, 0, KO)
            for n in range(NH):
                issue_wd(0, n)
            for e in range(1, E_PER):
                for m in range(MO):
                    issue_w(wgp, wg, "wg", e, m, 0, KO)
                for m in range(MO):
                    issue_w(wup, wu, "wu", e, m, 0, KO)
                for n in range(NH):
                    issue_wd(e, n)

            for e in range(E_PER):
                hT = hp.tile([P, MO, T], dt, tag="h")
                # all gate m-chunks first: the PE is gated only by the wg
                # stream, never by wu (which lands while these run)
                gss = []
                for m in range(MO):
                    pg = ps.tile([P, T], f32, tag="ps")
                    for k in range(KO):
                        nc.tensor.matmul(
                            pg[:],
                            wslice("wg", e, m, k),
                            xslice(k),
                            start=(k == 0),
                            stop=(k == KO - 1),
                        )
                    gs = gp.tile([P, T], f32, tag="g")
                    if sim_compat:
                        nc.scalar.activation(gs[:], pg[:], AF.Sigmoid)
                        nc.vector.tensor_tensor(
                            gs[:], gs[:], pg[:], mybir.AluOpType.mult
                        )
                    else:
                        nc.scalar.activation(gs[:], pg[:], AF.Silu)
                    gss.append(gs)
                for m in range(MO):
                    pu = ps.tile([P, T], f32, tag="ps")
                    for k in range(KO):
                        nc.tensor.matmul(
                            pu[:],
                            wslice("wu", e, m, k),
                            xslice(k),
                            start=(k == 0),
                            stop=(k == KO - 1),
                        )
                    nc.vector.tensor_tensor(
                        hT[:, m, :], gss[m][:], pu[:], mybir.AluOpType.mult
                    )
                for n in range(NH):
                    wdt = wdslabs[(e, n)]
                    for mt in range(TO):
                        last = e == E_PER - 1 and n == NH - 1 and mt == TO - 1
                        # the final group is stored as two half-width
                        # slices so the last HBM write receipt is smaller
                        nsub = 2 if last else 1
                        sub = NS // nsub
                        for s in range(nsub):
                            po = ps.tile([P, sub], f32, tag="ps")
                            for k in range(MO):
                                nc.tensor.matmul(
                                    po[:],
                                    hT[:, k, mt * P : (mt + 1) * P],
                                    wdt[:, k, s * sub : (s + 1) * sub],
                                    start=(k == 0),
                                    stop=(k == MO - 1),
                                )
                            ot = outp.tile([P, sub], f32, tag="o")
                            nc.vector.tensor_copy(ot[:], po[:])
                            nc.scalar.dma_start(
                                out[e, mt, :, n * NS + s * sub : n * NS + (s + 1) * sub],
                                ot[:],
                            )

    nc.compile()
    return nc


def get_program(sim_compat=False):
    key = ("nc", sim_compat)
    if key not in _CACHE:
        _CACHE[key] = _build_program(sim_compat=sim_compat)
    return _CACHE[key]


def _prep_in_maps(hidden_states, w_gate, w_up, w_down):
    bf16 = ml_dtypes.bfloat16
    x = np.asarray(hidden_states, dtype=np.float32)
    wg = np.asarray(w_gate, dtype=np.float32)
    wu = np.asarray(w_up, dtype=np.float32)
    wd = np.asarray(w_down, dtype=np.float32)

    # xT: [H, T] -> [128, KO, T], partition p + chunk k <-> H index k*128+p
    xt = np.ascontiguousarray(
        x.T.reshape(KO, P, T).transpose(1, 0, 2).astype(bf16)
    )
    # w_gate/w_up: [E, I, H] -> m-major slabs [E, MO, P, KO, 128]:
    #   slab[e, m, p, k, c] = W.T[k*128+p, m*128+c] = w[e, m*128+c, k*128+p]
    wgt = np.ascontiguousarray(
        wg.reshape(E, MO, P, KO, P).transpose(0, 1, 4, 3, 2).astype(bf16)
    )
    wut = np.ascontiguousarray(
        wu.reshape(E, MO, P, KO, P).transpose(0, 1, 4, 3, 2).astype(bf16)
    )
    # w_down: [E, H, I] -> n-major slabs [E, NH, P, MO, NS]:
    #   slab[e, n, p, k, c] = W.T[k*128+p, n*512+c] = w[e, n*512+c, k*128+p]
    wdt = np.ascontiguousarray(
        wd.reshape(E, NH, NS, MO, P).transpose(0, 1, 4, 3, 2).astype(bf16)
    )

    in_maps = []
    for c in range(N_CORES):
        sl = slice(c * E_PER, (c + 1) * E_PER)
        in_maps.append(
            {
                "xT": xt,
                "wg": np.ascontiguousarray(wgt[sl]),
                "wu": np.ascontiguousarray(wut[sl]),
                "wd": np.ascontiguousarray(wdt[sl]),
            }
        )
    return in_maps


def kernel(hidden_states, w_gate, w_up, w_down, _trace=False, _trace_kwargs=None):
    from concourse.bass_utils import run_bass_kernel_spmd

    nc = get_program()
    in_maps = _prep_in_maps(hidden_states, w_gate, w_up, w_down)
    kwargs = {}
    if _trace:
        kwargs = dict(trace=True, **(_trace_kwargs or {}))
    res = run_bass_kernel_spmd(nc, in_maps, core_ids=list(range(N_CORES)), **kwargs)
    out = np.concatenate(
        [res.results[c]["out"].reshape(E_PER * T, H) for c in range(N_CORES)], axis=0
    )
    if _trace:
        _CACHE["last_results"] = res
    return out


# revision 7
# speedup vs baseline: 1.0452x; 1.0349x over previous
"""DeepseekV3 MoE experts kernel for 8 Trainium2 NeuronCores.

Problem: every expert processes the FULL token set.
  g = x @ w_gate[e].T ; u = x @ w_up[e].T ; h = silu(g)*u
  out[e] = h @ w_down[e].T ;  concat over e -> [E*T, H]

Sharding: expert-parallel. Core c owns experts {2c, 2c+1}; hidden_states is
replicated; outputs are concatenated host-side (no on-device collectives).

Per-core compute (per expert e, with xT = x.T resident in SBUF):
  phase 1: gT[m*128:(m+1)*128, :] = wg_slab[m][:,k,:].T @ xT[:,k,:] (acc over k)
  phase 2: same for uT; hT = silu(gT) * uT  (in [I, T] layout, no transposes)
  phase 3: out[mt*128.., nslice] = hT[:,k,mtslice].T @ wd_slab[n][:,k,:]

All matmul operands are bf16 (host-cast); PSUM accumulation is fp32 and the
output is written back in fp32.

DMA plan (the mid-kernel PE stream is already at roofline; the wins are at
the edges):
  - Weights are laid out host-side in consumption-order slabs: wg/wu as
    [E, MO, 128, KO, 128] m-major slabs (0.5 MiB each; phase-1 m-chunk m
    needs only slab m), wd as [E, NH, 128, MO, 512] n-major slabs (1 MiB).
    The very first slab is split so the first matmul is gated on ~160 KiB.
  - All weight DMAs go on the Sync HWDGE ring in exact consumption order.
  - xT and all output stores go on the Scalar HWDGE ring (the second
    physical HWDGE ring), so the x ramp runs in parallel with the weight
    ramp and output stores never head-of-line block weight prefetches.
  - PSUM->SBUF output copies run on the (otherwise idle) Vector engine.
  - The last output group is split in two so the final store's HBM write
    receipt (which gates the kernel drain) covers half the bytes.

Self-contained: shapes hardcoded; inputs are the full arrays from
setup_inputs(); returns the full [4096, 2048] fp32 output.
"""

import numpy as np
import ml_dtypes

E, T, H, I = 16, 256, 2048, 1024
N_CORES = 8
E_PER = E // N_CORES  # 2
P = 128
KO = H // P  # 16 k-chunks for phases 1/2
MO = I // P  # 8 m-chunks for phases 1/2 (= k-chunks for phase 3)
TO = T // P  # 2 m-chunks for phase 3
NS = 512  # n-slice width for phase 3
NH = H // NS  # 4

WARMUP_MM = 40  # matmuls on a scratch tile bridging the preamble (~7us) to
# the point where the input stream is deep enough for a gapless phase 1
# (~13us). The HAM clock gate flips to 8/8 ~3.4us in, so the real matmul
# stream starts warm and never stalls long enough to re-throttle.

_CACHE: dict = {}


def _build_program(sim_compat=False):
    # sim_compat: CoreSim lacks the Silu LUT — express silu as
    # sigmoid(g)*g with an extra DVE multiply. HW uses the fused Silu op.
    import concourse.mybir as mybir
    import concourse.tile as tile
    from concourse import bacc

    dt = mybir.dt.bfloat16
    f32 = mybir.dt.float32
    AF = mybir.ActivationFunctionType

    nc = bacc.Bacc(None, target_bir_lowering=False, debug=False)

    xT = nc.dram_tensor("xT", [P, KO, T], dt, kind="ExternalInput")[:]
    wg = nc.dram_tensor("wg", [E_PER, MO, P, KO, P], dt, kind="ExternalInput")[:]
    wu = nc.dram_tensor("wu", [E_PER, MO, P, KO, P], dt, kind="ExternalInput")[:]
    wd = nc.dram_tensor("wd", [E_PER, NH, P, MO, NS], dt, kind="ExternalInput")[:]
    out = nc.dram_tensor("out", [E_PER, TO, P, H], f32, kind="ExternalOutput")[:]

    with tile.TileContext(nc) as tc:
        with (
            tc.tile_pool(name="xp", bufs=1) as xp,
            tc.tile_pool(name="wgp", bufs=10) as wgp,
            tc.tile_pool(name="wup", bufs=9) as wup,
            tc.tile_pool(name="wdp", bufs=6) as wdp,
            tc.tile_pool(name="hp", bufs=2) as hp,
            tc.tile_pool(name="gp", bufs=8) as gp,
            tc.tile_pool(name="op", bufs=8) as outp,
            tc.tile_pool(name="ps", bufs=8, space="PSUM") as ps,
        ):
            # PE warm-up on a zeroed scratch tile: flips the HAM clock gate
            # toward 8/8 while the first input DMAs are still in flight.
            warm = xp.tile([P, T], dt, tag="warm")
            nc.vector.memset(warm[:], 0.0)
            wps = ps.tile([P, T], f32, tag="ps")
            for _ in range(WARMUP_MM):
                nc.tensor.matmul(wps[:], warm[:, :P], warm[:], start=True, stop=True)

            xtiles = []  # (k0, kq, tile)
            wslabs: dict = {}  # (which, e, m) -> list of (k0, kq, tile)
            wdslabs: dict = {}  # (e, n) -> tile

            def issue_x(k0, kq, tag):
                t_ = xp.tile([P, kq, T], dt, tag=tag)
                nc.scalar.dma_start(t_[:], xT[:, k0 : k0 + kq, :])
                xtiles.append((k0, kq, t_))

            def xslice(k):
                for k0, kq, t_ in xtiles:
                    if k0 <= k < k0 + kq:
                        return t_[:, k - k0, :]
                raise KeyError(k)

            def issue_w(pool, src, which, e, m, k0, kq):
                t_ = pool.tile([P, kq, P], dt, tag=pool.name)
                nc.sync.dma_start(t_[:], src[e, m, :, k0 : k0 + kq, :])
                wslabs.setdefault((which, e, m), []).append((k0, kq, t_))

            def wslice(which, e, m, k):
                for k0, kq, t_ in wslabs[(which, e, m)]:
                    if k0 <= k < k0 + kq:
                        return t_[:, k - k0, :]
                raise KeyError((which, e, m, k))

            def issue_wd(e, n):
                t_ = wdp.tile([P, MO, NS], dt, tag="wd")
                nc.sync.dma_start(t_[:], wd[e, n, :, :, :])
                wdslabs[(e, n)] = t_

            # DMA issue order == consumption order. Weights on the Sync
            # ring; x on the Scalar ring so both ramps run in parallel.
            # The leading pieces are small so the first matmul starts as
            # soon as ~160 KiB has landed.
            issue_x(0, 8, "xa")
            issue_w(wgp, wg, "wg", 0, 0, 0, KO)
            issue_x(8, 8, "xb")
            for m in range(1, MO):
                issue_w(wgp, wg, "wg", 0, m, 0, KO)
            for m in range(MO):
                issue_w(wup, wu, "wu", 0, m, 0, KO)
            for n in range(NH):
                issue_wd(0, n)
            for e in range(1, E_PER):
                for m in range(MO):
                    issue_w(wgp, wg, "wg", e, m, 0, KO)
                for m in range(MO):
                    issue_w(wup, wu, "wu", e, m, 0, KO)
                for n in range(NH):
                    issue_wd(e, n)

            for e in range(E_PER):
                hT = hp.tile([P, MO, T], dt, tag="h")
                # all gate m-chunks first: the PE is gated only by the wg
                # stream, never by wu (which lands while these run)
                gss = []
                for m in range(MO):
                    pg = ps.tile([P, T], f32, tag="ps")
                    for k in range(KO):
                        nc.tensor.matmul(
                            pg[:],
                            wslice("wg", e, m, k),
                            xslice(k),
                            start=(k == 0),
                            stop=(k == KO - 1),
                        )
                    gs = gp.tile([P, T], f32, tag="g")
                    if sim_compat:
                        nc.scalar.activation(gs[:], pg[:], AF.Sigmoid)
                        nc.vector.tensor_tensor(
                            gs[:], gs[:], pg[:], mybir.AluOpType.mult
                        )
                    else:
                        nc.scalar.activation(gs[:], pg[:], AF.Silu)
                    gss.append(gs)
                for m in range(MO):
                    pu = ps.tile([P, T], f32, tag="ps")
                    for k in range(KO):
                        nc.tensor.matmul(
                            pu[:],
                            wslice("wu", e, m, k),
                            xslice(k),
                            start=(k == 0),
                            stop=(k == KO - 1),
                        )
                    nc.vector.tensor_tensor(
                        hT[:, m, :], gss[m][:], pu[:], mybir.AluOpType.mult
                    )
                for n in range(NH):
                    wdt = wdslabs[(e, n)]
                    for mt in range(TO):
                        last = e == E_PER - 1 and n == NH - 1 and mt == TO - 1
                        # the final group is stored as two half-width
                        # slices so the last HBM write receipt is smaller
                        nsub = 2 if last else 1
                        sub = NS // nsub
                        for s in range(nsub):
                            po = ps.tile([P, sub], f32, tag="ps")
                            for k in range(MO):
                                nc.tensor.matmul(
                                    po[:],
                                    hT[:, k, mt * P : (mt + 1) * P],
                                    wdt[:, k, s * sub : (s + 1) * sub],
                                    start=(k == 0),
                                    stop=(k == MO - 1),
                                )
                            ot = outp.tile([P, sub], f32, tag="o")
                            nc.vector.tensor_copy(ot[:], po[:])
                            nc.scalar.dma_start(
                                out[e, mt, :, n * NS + s * sub : n * NS + (s + 1) * sub],
                                ot[:],
                            )

    nc.compile()
    return nc


def get_program(sim_compat=False):
    key = ("nc", sim_compat)
    if key not in _CACHE:
        _CACHE[key] = _build_program(sim_compat=sim_compat)
    return _CACHE[key]


def _prep_in_maps(hidden_states, w_gate, w_up, w_down):
    bf16 = ml_dtypes.bfloat16
    x = np.asarray(hidden_states, dtype=np.float32)
    wg = np.asarray(w_gate, dtype=np.float32)
    wu = np.asarray(w_up, dtype=np.float32)
    wd = np.asarray(w_down, dtype=np.float32)

    # xT: [H, T] -> [128, KO, T], partition p + chunk k <-> H index k*128+p
    xt = np.ascontiguousarray(
        x.T.reshape(KO, P, T).transpose(1, 0, 2).astype(bf16)
    )
    # w_gate/w_up: [E, I, H] -> m-major slabs [E, MO, P, KO, 128]:
    #   slab[e, m, p, k, c] = W.T[k*128+p, m*128+c] = w[e, m*128+c, k*128+p]
    wgt = np.ascontiguousarray(
        wg.reshape(E, MO, P, KO, P).transpose(0, 1, 4, 3, 2).astype(bf16)
    )
    wut = np.ascontiguousarray(
        wu.reshape(E, MO, P, KO, P).transpose(0, 1, 4, 3, 2).astype(bf16)
    )
    # w_down: [E, H, I] -> n-major slabs [E, NH, P, MO, NS]:
    #   slab[e, n, p, k, c] = W.T[k*128+p, n*512+c] = w[e, n*512+c, k*128+p]
    wdt = np.ascontiguousarray(
        wd.reshape(E, NH, NS, MO, P).transpose(0, 1, 4, 3, 2).astype(bf16)
    )

    in_maps = []
    for c in range(N_CORES):
        sl = slice(c * E_PER, (c + 1) * E_PER)
        in_maps.append(
            {
                "xT": xt,
                "wg": np.ascontiguousarray(wgt[sl]),
                "wu": np.ascontiguousarray(wut[sl]),
                "wd": np.ascontiguousarray(wdt[sl]),
            }
        )
    return in_maps


def kernel(hidden_states, w_gate, w_up, w_down, _trace=False, _trace_kwargs=None):
    from concourse.bass_utils import run_bass_kernel_spmd

    nc = get_program()
    in_maps = _prep_in_maps(hidden_states, w_gate, w_up, w_down)
    kwargs = {}
    if _trace:
        kwargs = dict(trace=True, **(_trace_kwargs or {}))
    res = run_bass_kernel_spmd(nc, in_maps, core_ids=list(range(N_CORES)), **kwargs)
    out = np.concatenate(
        [res.results[c]["out"].reshape(E_PER * T, H) for c in range(N_CORES)], axis=0
    )
    if _trace:
        _CACHE["last_results"] = res
    return out


# revision 9
# speedup vs baseline: 1.0740x; 1.0276x over previous
"""DeepseekV3 MoE experts kernel for 8 Trainium2 NeuronCores.

Problem: every expert processes the FULL token set.
  g = x @ w_gate[e].T ; u = x @ w_up[e].T ; h = silu(g)*u
  out[e] = h @ w_down[e].T ;  concat over e -> [E*T, H]

Sharding: expert-parallel. Core c owns experts {2c, 2c+1}; hidden_states is
replicated; outputs are concatenated host-side (no on-device collectives).

Per-core compute (per expert e, with xT = x.T resident in SBUF):
  phase 1: gT[m*128:(m+1)*128, :] = wg_slab[m][:,k,:].T @ xT[:,k,:] (acc over k)
  phase 2: same for uT; hT = silu(gT) * uT  (in [I, T] layout, no transposes)
  phase 3: out[mt*128.., nslice] = hT[:,k,mtslice].T @ wd_slab[n][:,k,:]

All matmul operands are bf16 (host-cast); PSUM accumulation is fp32 and the
output is written back in fp32.

DMA plan (the mid-kernel PE stream is already at roofline; the wins are at
the edges):
  - Weights are laid out host-side in consumption-order slabs: wg/wu as
    [E, MO, 128, KO, 128] m-major slabs (0.5 MiB each; phase-1 m-chunk m
    needs only slab m), wd as [E, NH, 128, MO, 512] n-major slabs (1 MiB).
    The very first slab is split so the first matmul is gated on ~160 KiB.
  - All weight DMAs go on the Sync HWDGE ring in exact consumption order.
  - xT and all output stores go on the Scalar HWDGE ring (the second
    physical HWDGE ring), so the x ramp runs in parallel with the weight
    ramp and output stores never head-of-line block weight prefetches.
  - PSUM->SBUF output copies run on the (otherwise idle) Vector engine.
  - The last output group is split in two so the final store's HBM write
    receipt (which gates the kernel drain) covers half the bytes.

Self-contained: shapes hardcoded; inputs are the full arrays from
setup_inputs(); returns the full [4096, 2048] fp32 output.
"""

import numpy as np
import ml_dtypes

E, T, H, I = 16, 256, 2048, 1024
N_CORES = 8
E_PER = E // N_CORES  # 2
P = 128
KO = H // P  # 16 k-chunks for phases 1/2
MO = I // P  # 8 m-chunks for phases 1/2 (= k-chunks for phase 3)
TO = T // P  # 2 m-chunks for phase 3
NS = 512  # n-slice width for phase 3
NH = H // NS  # 4

WARMUP_MM = 40  # matmuls on a scratch tile bridging the preamble (~7us) to
# the point where the input stream is deep enough for a gapless phase 1
# (~13us). The HAM clock gate flips to 8/8 ~3.4us in, so the real matmul
# stream starts warm and never stalls long enough to re-throttle.

_CACHE: dict = {}


def _build_program(sim_compat=False):
    # sim_compat: CoreSim lacks the Silu LUT — express silu as
    # sigmoid(g)*g with an extra DVE multiply. HW uses the fused Silu op.
    import concourse.mybir as mybir
    import concourse.tile as tile
    from concourse import bacc

    dt = mybir.dt.bfloat16
    f32 = mybir.dt.float32
    AF = mybir.ActivationFunctionType

    nc = bacc.Bacc(None, target_bir_lowering=False, debug=False)

    xT = nc.dram_tensor("xT", [P, KO, T], dt, kind="ExternalInput")[:]
    wg = nc.dram_tensor("wg", [E_PER, MO, P, KO, P], dt, kind="ExternalInput")[:]
    wu = nc.dram_tensor("wu", [E_PER, MO, P, KO, P], dt, kind="ExternalInput")[:]
    wd = nc.dram_tensor("wd", [E_PER, NH, P, MO, NS], dt, kind="ExternalInput")[:]
    out = nc.dram_tensor("out", [E_PER, TO, P, H], f32, kind="ExternalOutput")[:]

    with tile.TileContext(nc) as tc:
        with (
            tc.tile_pool(name="xp", bufs=1) as xp,
            tc.tile_pool(name="wgp", bufs=12) as wgp,
            tc.tile_pool(name="wup", bufs=9) as wup,
            tc.tile_pool(name="wdp", bufs=6) as wdp,
            tc.tile_pool(name="hp", bufs=2) as hp,
            tc.tile_pool(name="gp", bufs=8) as gp,
            tc.tile_pool(name="op", bufs=8) as outp,
            tc.tile_pool(name="ps", bufs=8, space="PSUM") as ps,
        ):
            # PE warm-up on a zeroed scratch tile: flips the HAM clock gate
            # toward 8/8 while the first input DMAs are still in flight.
            warm = xp.tile([P, T], dt, tag="warm")
            nc.vector.memset(warm[:], 0.0)
            wps = ps.tile([P, T], f32, tag="ps")
            for _ in range(WARMUP_MM):
                nc.tensor.matmul(wps[:], warm[:, :P], warm[:], start=True, stop=True)

            xtiles = []  # (k0, kq, tile)
            wslabs: dict = {}  # (which, e, m) -> list of (k0, kq, tile)
            wdslabs: dict = {}  # (e, n) -> tile

            def issue_x(k0, kq, tag):
                t_ = xp.tile([P, kq, T], dt, tag=tag)
                nc.scalar.dma_start(t_[:], xT[:, k0 : k0 + kq, :])
                xtiles.append((k0, kq, t_))

            def xslice(k):
                for k0, kq, t_ in xtiles:
                    if k0 <= k < k0 + kq:
                        return t_[:, k - k0, :]
                raise KeyError(k)

            def issue_w(pool, src, which, e, m, k0, kq):
                t_ = pool.tile([P, kq, P], dt, tag=pool.name)
                nc.sync.dma_start(t_[:], src[e, m, :, k0 : k0 + kq, :])
                wslabs.setdefault((which, e, m), []).append((k0, kq, t_))

            def wslice(which, e, m, k):
                for k0, kq, t_ in wslabs[(which, e, m)]:
                    if k0 <= k < k0 + kq:
                        return t_[:, k - k0, :]
                raise KeyError((which, e, m, k))

            def issue_wd(e, n):
                t_ = wdp.tile([P, MO, NS], dt, tag="wd")
                nc.sync.dma_start(t_[:], wd[e, n, :, :, :])
                wdslabs[(e, n)] = t_

            # DMA issue order == consumption order. Weights on the Sync
            # ring; x on the Scalar ring so both ramps run in parallel.
            # The leading pieces are small so the first matmul starts as
            # soon as ~160 KiB has landed.
            issue_x(0, 8, "xa")
            issue_w(wgp, wg, "wg", 0, 0, 0, KO)
            issue_x(8, 8, "xb")
            # first slabs split into k-halves: each m-loop can start on the
            # first half while the second is still in flight (earlier sems
            # during the DMA ramp)
            for m in range(1, 4):
                issue_w(wgp, wg, "wg", 0, m, 0, KO // 2)
                issue_w(wgp, wg, "wg", 0, m, KO // 2, KO // 2)
            for m in range(4, MO):
                issue_w(wgp, wg, "wg", 0, m, 0, KO)
            for m in range(MO):
                issue_w(wup, wu, "wu", 0, m, 0, KO)
            for n in range(NH):
                issue_wd(0, n)
            for e in range(1, E_PER):
                for m in range(MO):
                    issue_w(wgp, wg, "wg", e, m, 0, KO)
                for m in range(MO):
                    issue_w(wup, wu, "wu", e, m, 0, KO)
                for n in range(NH):
                    issue_wd(e, n)

            for e in range(E_PER):
                hT = hp.tile([P, MO, T], dt, tag="h")
                # all gate m-chunks first: the PE is gated only by the wg
                # stream, never by wu (which lands while these run)
                gss = []
                for m in range(MO):
                    pg = ps.tile([P, T], f32, tag="ps")
                    for k in range(KO):
                        nc.tensor.matmul(
                            pg[:],
                            wslice("wg", e, m, k),
                            xslice(k),
                            start=(k == 0),
                            stop=(k == KO - 1),
                        )
                    gs = gp.tile([P, T], f32, tag="g")
                    if sim_compat:
                        nc.scalar.activation(gs[:], pg[:], AF.Sigmoid)
                        nc.vector.tensor_tensor(
                            gs[:], gs[:], pg[:], mybir.AluOpType.mult
                        )
                    else:
                        nc.scalar.activation(gs[:], pg[:], AF.Silu)
                    gss.append(gs)
                for m in range(MO):
                    pu = ps.tile([P, T], f32, tag="ps")
                    for k in range(KO):
                        nc.tensor.matmul(
                            pu[:],
                            wslice("wu", e, m, k),
                            xslice(k),
                            start=(k == 0),
                            stop=(k == KO - 1),
                        )
                    nc.vector.tensor_tensor(
                        hT[:, m, :], gss[m][:], pu[:], mybir.AluOpType.mult
                    )
                for n in range(NH):
                    wdt = wdslabs[(e, n)]
                    for mt in range(TO):
                        last = e == E_PER - 1 and n == NH - 1 and mt == TO - 1
                        # the final group is stored as two half-width
                        # slices so the last HBM write receipt is smaller
                        nsub = 2 if last else 1
                        sub = NS // nsub
                        for s in range(nsub):
                            po = ps.tile([P, sub], f32, tag="ps")
                            for k in range(MO):
                                nc.tensor.matmul(
                                    po[:],
                                    hT[:, k, mt * P : (mt + 1) * P],
                                    wdt[:, k, s * sub : (s + 1) * sub],
                                    start=(k == 0),
                                    stop=(k == MO - 1),
                                )
                            ot = outp.tile([P, sub], f32, tag="o")
                            nc.vector.tensor_copy(ot[:], po[:])
                            nc.scalar.dma_start(
                                out[e, mt, :, n * NS + s * sub : n * NS + (s + 1) * sub],
                                ot[:],
                            )

    nc.compile()
    return nc


def get_program(sim_compat=False):
    key = ("nc", sim_compat)
    if key not in _CACHE:
        _CACHE[key] = _build_program(sim_compat=sim_compat)
    return _CACHE[key]


def _prep_in_maps(hidden_states, w_gate, w_up, w_down):
    bf16 = ml_dtypes.bfloat16
    x = np.asarray(hidden_states, dtype=np.float32)
    wg = np.asarray(w_gate, dtype=np.float32)
    wu = np.asarray(w_up, dtype=np.float32)
    wd = np.asarray(w_down, dtype=np.float32)

    # xT: [H, T] -> [128, KO, T], partition p + chunk k <-> H index k*128+p
    xt = np.ascontiguousarray(
        x.T.reshape(KO, P, T).transpose(1, 0, 2).astype(bf16)
    )
    # w_gate/w_up: [E, I, H] -> m-major slabs [E, MO, P, KO, 128]:
    #   slab[e, m, p, k, c] = W.T[k*128+p, m*128+c] = w[e, m*128+c, k*128+p]
    wgt = np.ascontiguousarray(
        wg.reshape(E, MO, P, KO, P).transpose(0, 1, 4, 3, 2).astype(bf16)
    )
    wut = np.ascontiguousarray(
        wu.reshape(E, MO, P, KO, P).transpose(0, 1, 4, 3, 2).astype(bf16)
    )
    # w_down: [E, H, I] -> n-major slabs [E, NH, P, MO, NS]:
    #   slab[e, n, p, k, c] = W.T[k*128+p, n*512+c] = w[e, n*512+c, k*128+p]
    wdt = np.ascontiguousarray(
        wd.reshape(E, NH, NS, MO, P).transpose(0, 1, 4, 3, 2).astype(bf16)
    )

    in_maps = []
    for c in range(N_CORES):
        sl = slice(c * E_PER, (c + 1) * E_PER)
        in_maps.append(
            {
                "xT": xt,
                "wg": np.ascontiguousarray(wgt[sl]),
                "wu": np.ascontiguousarray(wut[sl]),
                "wd": np.ascontiguousarray(wdt[sl]),
            }
        )
    return in_maps


def kernel(hidden_states, w_gate, w_up, w_down, _trace=False, _trace_kwargs=None):
    from concourse.bass_utils import run_bass_kernel_spmd

    nc = get_program()
    in_maps = _prep_in_maps(hidden_states, w_gate, w_up, w_down)
    kwargs = {}
    if _trace:
        kwargs = dict(trace=True, **(_trace_kwargs or {}))
    res = run_bass_kernel_spmd(nc, in_maps, core_ids=list(range(N_CORES)), **kwargs)
    out = np.concatenate(
        [res.results[c]["out"].reshape(E_PER * T, H) for c in range(N_CORES)], axis=0
    )
    if _trace:
        _CACHE["last_results"] = res
    return out
